# revision 11
# baseline (speedup 1.0000x reference)
"""Self-contained Trainium2 kernel for nn_CASTGModel_39608188404339.

Host: embedding gather, boundary detection, segment pooling, upsample, loss.
Device (8 NeuronCores, SPMD): the 4-layer jagged transformer over pooled
segments + gate + tied-embedding head.

Sharding: data-parallel over batch (4 elements) x 2-way sequence split per
element. Each core owns TQ = S_pad/2 query tokens of one batch element;
K/V are exchanged within the core pair via AllGather each layer.

Activations live transposed in SBUF: [D on partitions (4x128), tokens free].
All matmuls use float32r (TF32) inputs with fp32 PSUM accumulation.
"""

import contextlib
import ctypes
import math
import sys
import types

import numpy as np


# ---------------------------------------------------------------- ntff shim
def _install_ntff_shim(so_path="/opt/axon/libaxon_pjrt.so"):
    try:
        import antenv.axon_hooks  # noqa: F401
        return
    except ImportError:
        pass
    try:
        lib = ctypes.CDLL(so_path)
    except OSError:
        return
    if not hasattr(lib, "axon_start_nrt_profile"):
        hook = None
    else:
        lib.axon_start_nrt_profile.argtypes = [ctypes.POINTER(ctypes.c_int64), ctypes.c_size_t]
        lib.axon_start_nrt_profile.restype = ctypes.c_int64
        lib.axon_stop_nrt_profile.argtypes = [ctypes.c_char_p]
        lib.axon_stop_nrt_profile.restype = ctypes.c_int64

        @contextlib.contextmanager
        def hook(output_dir, device_ids):
            import jax
            jax.devices()
            if device_ids:
                ids = (ctypes.c_int64 * len(device_ids))(*device_ids)
                rc = lib.axon_start_nrt_profile(ids, len(device_ids))
            else:
                rc = lib.axon_start_nrt_profile(None, 0)
            if rc != 0:
                raise RuntimeError(f"axon_start_nrt_profile rc={rc}")
            try:
                yield
            finally:
                n = lib.axon_stop_nrt_profile(str(output_dir).encode())
                print(f"ntff profile: {n} file(s) written to {output_dir}", file=sys.stderr)

    mod = types.ModuleType("antenv.axon_hooks")
    mod.get_axon_ntff_profile_hook = lambda: hook
    mod.set_axon_ntff_profile_hook = lambda h: None
    import antenv
    antenv.axon_hooks = mod
    sys.modules["antenv.axon_hooks"] = mod


_install_ntff_shim()

import concourse.mybir as mybir  # noqa: E402
import concourse.tile as tile  # noqa: E402
from concourse import bacc  # noqa: E402
from concourse.bass_utils import run_bass_kernel_spmd  # noqa: E402

f32 = mybir.dt.float32
f32r = mybir.dt.float32r
AF = mybir.ActivationFunctionType
ALU = mybir.AluOpType

B, T, V = 4, 2048, 256
D, H, L = 512, 8, 4
DH, FF = D // H, 4 * D
DC = D // 128          # 4 d-chunks
FC = FF // 128         # 16 ff-chunks
NCORES = 8
MASK_BIAS = -60.0

LAST_EXEC_NS = None
_NC_CACHE = {}


def tf32_round(x):
    u = np.ascontiguousarray(x, dtype=np.float32).view(np.uint32)
    r = (u + np.uint32(0x0FFF) + ((u >> np.uint32(13)) & np.uint32(1))) & np.uint32(0xFFFFE000)
    return r.view(np.float32)


def _chunks(total, step=128):
    return [(o, min(step, total - o)) for o in range(0, total, step)]


# ---------------------------------------------------------------- builder
def build(S_pad):
    KC = S_pad // 128
    TQ = S_pad // 2
    HN = TQ // 2
    HALVES = [(0, HN), (HN, TQ - HN)]
    TCS = _chunks(TQ)

    nc = bacc.Bacc("TRN2", target_bir_lowering=False, debug=False, num_devices=NCORES)

    def din(name, shape, dt=f32):
        return nc.dram_tensor(name, shape, dt, kind="ExternalInput").ap()

    zt_d = din("zt", [D, TQ])
    maskb_d = din("maskb", [128, KC])
    wq_d = din("wq", [L, D, D], f32r)
    wk_d = din("wk", [L, D, D], f32r)
    wv_d = din("wv", [L, D, D], f32r)
    bq_d = din("bq", [L, 128, DC])
    bk_d = din("bk", [L, 128, DC])
    bvrep_d = din("bvrep", [L, 128, D])
    wo_d = din("wo", [L, D, D], f32r)
    bo_d = din("bo", [L, 128, DC])
    w1_d = din("w1", [L, D, FF], f32r)
    b1_d = din("b1", [L, 128, FC])
    w2_d = din("w2", [L, FF, D], f32r)
    b2_d = din("b2", [L, 128, DC])
    l1g_d = din("l1g", [L, 128, DC])
    l1b_d = din("l1b", [L, 128, DC])
    l2g_d = din("l2g", [L, 128, DC])
    l2b_d = din("l2b", [L, 128, DC])
    lnfg_d = din("lnfg", [128, DC])
    lnfb_d = din("lnfb", [128, DC])
    embt_d = din("embt", [D, V], f32r)
    bd_d = din("bd", [128, V // 128])
    wg_d = din("wg", [128, DC], f32r)
    bg_d = din("bg", [1, 1])

    lsegt_d = nc.dram_tensor("lsegt", [V, TQ], f32, kind="ExternalOutput").ap()
    gate_d = nc.dram_tensor("gate", [1, TQ], f32, kind="ExternalOutput").ap()

    with tile.TileContext(nc) as tc:
        with (
            tc.tile_pool(name="res", bufs=1) as res,
            tc.tile_pool(name="vec", bufs=2) as vec,
            tc.tile_pool(name="wp", bufs=6) as wp,
            tc.tile_pool(name="sc", bufs=2) as sc,
            tc.tile_pool(name="ex", bufs=4) as ex,
            tc.tile_pool(name="pp", bufs=2, space="PSUM") as pp,
            tc.tile_pool(name="dr", bufs=2, space="DRAM") as dr,
        ):
            # persistent tiles
            xt = res.tile([128, DC * TQ], f32, tag="xt")
            xr = res.tile([128, DC * TQ], f32r, tag="xr")
            xn = res.tile([128, DC * TQ], f32r, tag="xn")
            qt = res.tile([128, DC * TQ], f32r, tag="qt")
            ktown = res.tile([128, DC * TQ], f32r, tag="ktown")
            ktf = res.tile([128, DC * S_pad], f32r, tag="ktf")
            vext = res.tile([128, KC * 520], f32r, tag="vext")
            vown = res.tile([128, len(TCS) * 520], f32r, tag="vown")
            ot = res.tile([128, DC * TQ], f32r, tag="ot")
            gb = res.tile([128, TQ], f32, tag="gb")
            lsegs = res.tile([128, (V // 128) * TQ], f32, tag="lsegs")
            maskt = res.tile([128, KC], f32, tag="maskt")
            ones = res.tile([128, 1], f32r, tag="ones")
            epst = res.tile([1, 1], f32, tag="epst")

            nc.vector.memset(ones[:].bitcast(f32), 1.0)
            nc.vector.memset(epst[:], 1e-5)
            # ones columns of vown (written once; v writes only cols 0..63/head)
            for tc_i in range(len(TCS)):
                nc.vector.memset(
                    vown[:, tc_i * 520:(tc_i + 1) * 520]
                    .rearrange("p (h x) -> p h x", h=H)[:, :, DH].bitcast(f32),
                    1.0,
                )

            nc.sync.dma_start(maskt[:], maskb_d[:])
            for c in range(DC):
                nc.sync.dma_start(xt[:, c * TQ:(c + 1) * TQ], zt_d[c * 128:(c + 1) * 128, :])
                nc.scalar.copy(xr[:, c * TQ:(c + 1) * TQ], xt[:, c * TQ:(c + 1) * TQ])

            def ln(gvec, bvec):
                """LN over D (partition x chunks) of xt/xr -> xn (f32r)."""
                for ho, hl in HALVES:
                    sq = sc.tile([128, HN], f32r, tag="sq")
                    psm = pp.tile([128, HN], f32, tag="pq", name="psm")[0:1]
                    pss = pp.tile([128, HN], f32, tag="pq", name="pss")[0:1]
                    for c in range(DC):
                        xrc = xr[:, c * TQ + ho: c * TQ + ho + hl]
                        nc.vector.tensor_tensor(sq[:, :hl], xrc, xrc, ALU.mult)
                        nc.tensor.matmul(psm[:, :hl], ones[:], xrc,
                                         start=(c == 0), stop=(c == DC - 1))
                        nc.tensor.matmul(pss[:, :hl], ones[:], sq[:, :hl],
                                         start=(c == 0), stop=(c == DC - 1))
                    A = sc.tile([1, HN], f32, tag="lnA")
                    Bt = sc.tile([1, HN], f32, tag="lnB")
                    nc.vector.tensor_scalar_mul(A[:, :hl], psm[:, :hl], 1.0 / D)
                    nc.vector.tensor_scalar_mul(Bt[:, :hl], pss[:, :hl], 1.0 / D)
                    mub = sc.tile([128, HN], f32, tag="mub")
                    rstdb = sc.tile([128, HN], f32, tag="rstdb")
                    nc.gpsimd.partition_broadcast(mub[:, :hl], A[:, :hl])
                    # A = mu^2; B = var = ms - mu^2; A = sqrt(var+eps); B = rstd
                    nc.vector.tensor_tensor(A[:, :hl], A[:, :hl], A[:, :hl], ALU.mult)
                    nc.vector.tensor_tensor(Bt[:, :hl], Bt[:, :hl], A[:, :hl], ALU.subtract)
                    nc.scalar.activation(A[:, :hl], Bt[:, :hl], AF.Sqrt, bias=epst[:])
                    nc.vector.reciprocal(Bt[:, :hl], A[:, :hl])
                    nc.gpsimd.partition_broadcast(rstdb[:, :hl], Bt[:, :hl])
                    for c in range(DC):
                        s = slice(c * TQ + ho, c * TQ + ho + hl)
                        t1 = sc.tile([128, HN], f32, tag="t1")
                        nc.vector.tensor_tensor(t1[:, :hl], xt[:, s], mub[:, :hl], ALU.subtract)
                        nc.vector.scalar_tensor_tensor(
                            t1[:, :hl], t1[:, :hl], gvec[:, c:c + 1], rstdb[:, :hl],
                            ALU.mult, ALU.mult)
                        nc.vector.tensor_scalar_add(xn[:, s], t1[:, :hl], bvec[:, c:c + 1])

            for l in range(L):
                l1g = vec.tile([128, DC], f32, tag="l1g")
                l1b = vec.tile([128, DC], f32, tag="l1b")
                l2g = vec.tile([128, DC], f32, tag="l2g")
                l2b = vec.tile([128, DC], f32, tag="l2b")
                bqv = vec.tile([128, DC], f32, tag="bqv")
                bkv = vec.tile([128, DC], f32, tag="bkv")
                bov = vec.tile([128, DC], f32, tag="bov")
                b2v = vec.tile([128, DC], f32, tag="b2v")
                b1v = vec.tile([128, FC], f32, tag="b1v")
                bvr = vec.tile([128, D], f32, tag="bvr")
                for t_, d_ in ((l1g, l1g_d), (l1b, l1b_d), (l2g, l2g_d), (l2b, l2b_d),
                               (bqv, bq_d), (bkv, bk_d), (bov, bo_d), (b2v, b2_d),
                               (b1v, b1_d), (bvr, bvrep_d)):
                    nc.sync.dma_start(t_[:], d_[l])

                # ---- LN1
                ln(l1g, l1b)

                # ---- K projection (own tokens)
                for kind, wdram, bvec, dst in (("k", wk_d, bkv, ktown),):
                    wts = []
                    for c in range(DC):
                        wt = wp.tile([128, D], f32r, tag="wt")
                        nc.sync.dma_start(wt[:], wdram[l, c * 128:(c + 1) * 128, :])
                        wts.append(wt)
                    for mc in range(DC):
                        for ho, hl in HALVES:
                            pq = pp.tile([128, HN], f32, tag="pq")
                            for c in range(DC):
                                nc.tensor.matmul(
                                    pq[:, :hl], wts[c][:, mc * 128:(mc + 1) * 128],
                                    xn[:, c * TQ + ho: c * TQ + ho + hl],
                                    start=(c == 0), stop=(c == DC - 1))
                            nc.vector.tensor_scalar_add(
                                dst[:, mc * TQ + ho: mc * TQ + ho + hl],
                                pq[:, :hl], bvec[:, mc:mc + 1])

                # ---- V projection (own tokens) into vown (65-strided head layout)
                wvt = []
                for c in range(DC):
                    wt = wp.tile([128, D], f32r, tag="wt")
                    nc.sync.dma_start(wt[:], wv_d[l, c * 128:(c + 1) * 128, :])
                    wvt.append(wt)
                for ti, (toff, tlen) in enumerate(TCS):
                    pvt = pp.tile([128, D], f32, tag="pq", name="pvt")
                    for c in range(DC):
                        nc.tensor.matmul(
                            pvt[:tlen, :], xn[:, c * TQ + toff: c * TQ + toff + tlen],
                            wvt[c][:], start=(c == 0), stop=(c == DC - 1))
                    nc.vector.tensor_tensor(
                        vown[:, ti * 520:(ti + 1) * 520]
                        .rearrange("p (h x) -> p h x", h=H)[:tlen, :, 0:DH],
                        pvt[:tlen, :].rearrange("p (h x) -> p h x", h=H),
                        bvr[:tlen, :].rearrange("p (h x) -> p h x", h=H),
                        ALU.add)

                # ---- exchange K/V within the pair (single fused AllGather)
                KTN = D * TQ
                XN_ = KTN + TQ * 520
                x_src = dr.tile([XN_], f32, tag="x_src")
                x_dst = dr.tile([2, XN_], f32, tag="x_dst")
                for c in range(DC):
                    nc.sync.dma_start(
                        x_src[c * 128 * TQ:(c + 1) * 128 * TQ]
                        .rearrange("(p t) -> p t", p=128),
                        ktown[:, c * TQ:(c + 1) * TQ].bitcast(f32))
                for ti, (toff, tlen) in enumerate(TCS):
                    nc.sync.dma_start(
                        x_src[KTN + toff * 520: KTN + (toff + tlen) * 520]
                        .rearrange("(r x) -> r x", x=520),
                        vown[0:tlen, ti * 520:(ti + 1) * 520].bitcast(f32))
                nc.gpsimd.collective_compute(
                    "AllGather", ALU.bypass,
                    replica_groups=[[0, 1], [2, 3], [4, 5], [6, 7]],
                    ins=[x_src[:]], outs=[x_dst[:]])
                for c in range(DC):
                    for half in range(2):
                        nc.sync.dma_start(
                            ktf[:, c * S_pad + half * TQ: c * S_pad + (half + 1) * TQ]
                            .bitcast(f32),
                            x_dst[half, c * 128 * TQ:(c + 1) * 128 * TQ]
                            .rearrange("(p t) -> p t", p=128))
                for j in range(KC):
                    g0 = j * 128
                    while g0 < (j + 1) * 128:
                        half = g0 // TQ
                        r0 = g0 % TQ
                        seg = min((j + 1) * 128 - g0, TQ - r0)
                        p0 = g0 - j * 128
                        nc.sync.dma_start(
                            vext[p0:p0 + seg, j * 520:(j + 1) * 520].bitcast(f32),
                            x_dst[half, KTN + r0 * 520: KTN + (r0 + seg) * 520]
                            .rearrange("(r x) -> r x", x=520))
                        g0 += seg

                # ---- Q projection (overlaps the exchange)
                for kind, wdram, bvec, dst in (("q", wq_d, bqv, qt),):
                    wts = []
                    for c in range(DC):
                        wt = wp.tile([128, D], f32r, tag="wt", name="wt")
                        nc.sync.dma_start(wt[:], wdram[l, c * 128:(c + 1) * 128, :])
                        wts.append(wt)
                    for mc in range(DC):
                        for ho, hl in HALVES:
                            pq = pp.tile([128, HN], f32, tag="pq", name="pq")
                            for c in range(DC):
                                nc.tensor.matmul(
                                    pq[:, :hl], wts[c][:, mc * 128:(mc + 1) * 128],
                                    xn[:, c * TQ + ho: c * TQ + ho + hl],
                                    start=(c == 0), stop=(c == DC - 1))
                            nc.vector.tensor_scalar_add(
                                dst[:, mc * TQ + ho: mc * TQ + ho + hl],
                                pq[:, :hl], bvec[:, mc:mc + 1])

                # ---- attention
                att_cm = tc.tile_pool(name=f"att{l}", bufs=2, space="PSUM")
                att_ps = att_cm.__enter__()
                for h in range(H):
                    prow = (h % 2) * DH
                    cblk = h // 2
                    pos = [att_ps.tile([65, HN], f32, tag="po", name="po") for _ in range(2)]
                    for j in range(KC):
                        pS = att_ps.tile([128, 1024], f32, tag="pS", name="pS")
                        for hi, (ho, hl) in enumerate(HALVES):
                            nc.tensor.matmul(
                                pS[:, hi * 512: hi * 512 + hl],
                                ktf[prow:prow + DH, cblk * S_pad + j * 128: cblk * S_pad + (j + 1) * 128],
                                qt[prow:prow + DH, cblk * TQ + ho: cblk * TQ + ho + hl],
                                start=True, stop=True)
                        et = ex.tile([128, 2 * HN], f32r, tag="et")
                        nc.scalar.activation(
                            et[:].rearrange("p (b x) -> p b x", b=2),
                            pS[:].rearrange("p (b x) -> p b x", b=2)[:, :, 0:HN],
                            AF.Exp, bias=maskt[:, j:j + 1], scale=0.125)
                        for hi, (ho, hl) in enumerate(HALVES):
                            nc.tensor.matmul(
                                pos[hi][:, :hl],
                                vext[:, j * 520 + h * 65: j * 520 + (h + 1) * 65],
                                et[:, hi * HN: hi * HN + hl],
                                start=(j == 0), stop=(j == KC - 1))
                    for hi, (ho, hl) in enumerate(HALVES):
                        po = pos[hi]
                        rec = sc.tile([1, HN], f32, tag="rec")
                        nc.vector.reciprocal(rec[:, :hl], po[DH:DH + 1, :hl])
                        rb = sc.tile([64, HN], f32, tag="rb")
                        nc.gpsimd.partition_broadcast(rb[:, :hl], rec[:, :hl])
                        nc.vector.tensor_tensor(
                            ot[prow:prow + DH, cblk * TQ + ho: cblk * TQ + ho + hl],
                            po[0:DH, :hl], rb[:, :hl], ALU.mult)
                att_cm.__exit__(None, None, None)

                # ---- Wo + residual
                wos = []
                for c in range(DC):
                    wt = wp.tile([128, D], f32r, tag="wt")
                    nc.sync.dma_start(wt[:], wo_d[l, c * 128:(c + 1) * 128, :])
                    wos.append(wt)
                for mc in range(DC):
                    for ho, hl in HALVES:
                        pw = pp.tile([128, HN], f32, tag="pq")
                        for c in range(DC):
                            nc.tensor.matmul(
                                pw[:, :hl], wos[c][:, mc * 128:(mc + 1) * 128],
                                ot[:, c * TQ + ho: c * TQ + ho + hl],
                                start=(c == 0), stop=(c == DC - 1))
                        s = slice(mc * TQ + ho, mc * TQ + ho + hl)
                        nc.vector.scalar_tensor_tensor(
                            xt[:, s], pw[:, :hl], bov[:, mc:mc + 1], xt[:, s],
                            ALU.add, ALU.add)
                    nc.scalar.copy(xr[:, mc * TQ:(mc + 1) * TQ],
                                   xt[:, mc * TQ:(mc + 1) * TQ])

                # ---- LN2
                ln(l2g, l2b)

                # ---- FFN (fused FF1 -> gelu -> FF2 accumulation, per token half)
                ffn_cm = tc.tile_pool(name=f"ffn{l}", bufs=4, space="PSUM")
                ffn_ps = ffn_cm.__enter__()
                for ho, hl in HALVES:
                    accs = [ffn_ps.tile([128, HN], f32, tag="pacc", name=f"acc{mc}")
                            for mc in range(DC)]
                    for fc in range(FC):
                        w1ts = []
                        for c in range(DC):
                            w1t = wp.tile([128, 128], f32r, tag="w1t", name="w1t")
                            nc.sync.dma_start(
                                w1t[:], w1_d[l, c * 128:(c + 1) * 128,
                                             fc * 128:(fc + 1) * 128])
                            w1ts.append(w1t)
                        w2t = wp.tile([128, D], f32r, tag="w2t", name="w2t")
                        nc.sync.dma_start(w2t[:], w2_d[l, fc * 128:(fc + 1) * 128, :])
                        pf = pp.tile([128, HN], f32, tag="pq", name="pf")
                        for c in range(DC):
                            nc.tensor.matmul(
                                pf[:, :hl], w1ts[c][:],
                                xn[:, c * TQ + ho: c * TQ + ho + hl],
                                start=(c == 0), stop=(c == DC - 1))
                        fft = ex.tile([128, HN], f32r, tag="fft", name="fft")
                        nc.scalar.activation(fft[:, :hl], pf[:, :hl],
                                             AF.Gelu_apprx_tanh, bias=b1v[:, fc:fc + 1])
                        for mc in range(DC):
                            nc.tensor.matmul(
                                accs[mc][:, :hl], w2t[:, mc * 128:(mc + 1) * 128],
                                fft[:, :hl], start=(fc == 0), stop=(fc == FC - 1))
                    for mc in range(DC):
                        s_ = slice(mc * TQ + ho, mc * TQ + ho + hl)
                        nc.vector.scalar_tensor_tensor(
                            xt[:, s_], accs[mc][:, :hl], b2v[:, mc:mc + 1], xt[:, s_],
                            ALU.add, ALU.add)
                        if ho != 0:
                            nc.scalar.copy(xr[:, mc * TQ:(mc + 1) * TQ],
                                           xt[:, mc * TQ:(mc + 1) * TQ])
                ffn_cm.__exit__(None, None, None)

            # ---- head: gate
            wgt = vec.tile([128, DC], f32r, tag="wgt")
            bgt = vec.tile([1, 1], f32, tag="bgt")
            lnfg = vec.tile([128, DC], f32, tag="lnfg")
            lnfb = vec.tile([128, DC], f32, tag="lnfb")
            bdv = vec.tile([128, V // 128], f32, tag="bdv")
            nc.sync.dma_start(wgt[:], wg_d[:])
            nc.sync.dma_start(bgt[:], bg_d[:])
            nc.sync.dma_start(lnfg[:], lnfg_d[:])
            nc.sync.dma_start(lnfb[:], lnfb_d[:])
            nc.sync.dma_start(bdv[:], bd_d[:])
            gatet = res.tile([1, TQ], f32, tag="gatet")
            for ho, hl in HALVES:
                pg = pp.tile([128, HN], f32, tag="pq", name="pg")[0:1]
                for c in range(DC):
                    nc.tensor.matmul(pg[:, :hl], wgt[:, c:c + 1],
                                     xr[:, c * TQ + ho: c * TQ + ho + hl],
                                     start=(c == 0), stop=(c == DC - 1))
                nc.scalar.activation(gatet[:, ho:ho + hl], pg[:, :hl], AF.Sigmoid,
                                     bias=bgt[:])
            nc.sync.dma_start(gate_d[:], gatet[:])
            nc.gpsimd.partition_broadcast(gb[:], gatet[:])
            for c in range(DC):
                s = slice(c * TQ, (c + 1) * TQ)
                nc.vector.tensor_tensor(xt[:, s], xt[:, s], gb[:], ALU.mult)
                nc.scalar.copy(xr[:, s], xt[:, s])

            # ---- head: LNf + logits
            ln(lnfg, lnfb)
            embts = []
            for c in range(DC):
                wt = wp.tile([128, V], f32r, tag="embt")
                nc.sync.dma_start(wt[:], embt_d[c * 128:(c + 1) * 128, :])
                embts.append(wt)
            for vc in range(V // 128):
                for ho, hl in HALVES:
                    pl = pp.tile([128, HN], f32, tag="pq")
                    for c in range(DC):
                        nc.tensor.matmul(
                            pl[:, :hl], embts[c][:, vc * 128:(vc + 1) * 128],
                            xn[:, c * TQ + ho: c * TQ + ho + hl],
                            start=(c == 0), stop=(c == DC - 1))
                    nc.vector.tensor_scalar_add(
                        lsegs[:, vc * TQ + ho: vc * TQ + ho + hl], pl[:, :hl],
                        bdv[:, vc:vc + 1])
            for vc in range(V // 128):
                nc.sync.dma_start(lsegt_d[vc * 128:(vc + 1) * 128, :],
                                  lsegs[:, vc * TQ:(vc + 1) * TQ])

    nc.compile()
    return nc


def _get_nc(S_pad):
    if S_pad not in _NC_CACHE:
        _NC_CACHE[S_pad] = build(S_pad)
    return _NC_CACHE[S_pad]


# ---------------------------------------------------------------- host side
def _sigmoid64(x):
    x = x.astype(np.float64)
    return np.where(x >= 0, 1.0 / (1.0 + np.exp(-x)), np.exp(x) / (1.0 + np.exp(x)))


def kernel(idx, targets, params):
    global LAST_EXEC_NS
    idx = np.asarray(idx)
    targets = np.asarray(targets)
    p = {k: np.asarray(v, dtype=np.float32) for k, v in params.items()}

    embed = p["embed"]                      # [V, D]
    h = embed[idx]                          # [B, T, D] f32
    blogit = (h.reshape(-1, D) @ p["Wb"]).reshape(B, T) + p["bb"][0]
    psig = _sigmoid64(blogit / np.float32(0.5))
    hard = psig > 0.5
    hard[:, 0] = True
    n_segs = hard.sum(1).astype(np.int64)
    S = int(n_segs.max())
    S_pad = min(max(1152, 128 * math.ceil((S + 1) / 128)), 2048)
    KC = S_pad // 128
    TQ = S_pad // 2

    # segment-pool z
    z = np.zeros((B, S_pad, D), np.float32)
    for b in range(B):
        starts = np.flatnonzero(hard[b])
        sums = np.add.reduceat(h[b], starts, axis=0)
        cnts = np.diff(np.append(starts, T)).astype(np.float32)
        z[b, :n_segs[b]] = sums / cnts[:, None]

    # shared (replicated) weight tensors
    Wqkv, bqkv = p["Wqkv"], p["bqkv"]
    shared = {
        "wq": tf32_round(Wqkv[:, :, 0:D]),
        "wk": tf32_round(Wqkv[:, :, D:2 * D]),
        "wv": tf32_round(Wqkv[:, :, 2 * D:3 * D]),
        "bq": np.ascontiguousarray(bqkv[:, 0:D].reshape(L, DC, 128).transpose(0, 2, 1)),
        "bk": np.ascontiguousarray(bqkv[:, D:2 * D].reshape(L, DC, 128).transpose(0, 2, 1)),
        "bvrep": np.ascontiguousarray(
            np.broadcast_to(bqkv[:, None, 2 * D:3 * D], (L, 128, D))),
        "wo": tf32_round(p["Wo"]),
        "bo": np.ascontiguousarray(p["bo"].reshape(L, DC, 128).transpose(0, 2, 1)),
        "w1": tf32_round(p["W1"]),
        "b1": np.ascontiguousarray(p["b1"].reshape(L, FC, 128).transpose(0, 2, 1)),
        "w2": tf32_round(p["W2"]),
        "b2": np.ascontiguousarray(p["b2"].reshape(L, DC, 128).transpose(0, 2, 1)),
        "l1g": np.ascontiguousarray(p["ln1_g"].reshape(L, DC, 128).transpose(0, 2, 1)),
        "l1b": np.ascontiguousarray(p["ln1_b"].reshape(L, DC, 128).transpose(0, 2, 1)),
        "l2g": np.ascontiguousarray(p["ln2_g"].reshape(L, DC, 128).transpose(0, 2, 1)),
        "l2b": np.ascontiguousarray(p["ln2_b"].reshape(L, DC, 128).transpose(0, 2, 1)),
        "lnfg": np.ascontiguousarray(p["lnf_g"].reshape(DC, 128).T),
        "lnfb": np.ascontiguousarray(p["lnf_b"].reshape(DC, 128).T),
        "embt": tf32_round(np.ascontiguousarray(embed.T)),
        "bd": np.ascontiguousarray(p["bd"].reshape(V // 128, 128).T),
        "wg": tf32_round(np.ascontiguousarray(p["Wg"].reshape(DC, 128).T)),
        "bg": p["bg"].reshape(1, 1),
    }

    in_maps = []
    for c in range(NCORES):
        b = c // 2
        lo = (c % 2) * TQ
        m = dict(shared)
        m["zt"] = np.ascontiguousarray(z[b, lo:lo + TQ, :].T)
        kmask = np.where(np.arange(S_pad) < n_segs[b], 0.0, MASK_BIAS).astype(np.float32)
        m["maskb"] = np.ascontiguousarray(kmask.reshape(KC, 128).T)
        in_maps.append(m)

    nc = _get_nc(S_pad)
    import os
    trace = bool(os.environ.get("BASS_KERNEL_TRACE"))
    res = run_bass_kernel_spmd(nc, in_maps, list(range(NCORES)), trace=trace)
    if trace:
        LAST_EXEC_NS = res.exec_time_ns

    lseg = np.zeros((B, S_pad, V), np.float32)
    gate_dev = np.zeros((B, S_pad), np.float32)
    for c in range(NCORES):
        b = c // 2
        lo = (c % 2) * TQ
        lseg[b, lo:lo + TQ] = res.results[c]["lsegt"].T
        gate_dev[b, lo:lo + TQ] = res.results[c]["gate"][0]

    # upsample + loss (host, f64)
    imap = (np.arange(T) * S) // T
    logits = lseg[:, imap, :]
    ls64 = lseg[:, :S, :].astype(np.float64)
    ls64 = ls64 - ls64.max(-1, keepdims=True)
    ls64 = ls64 - np.log(np.exp(ls64).sum(-1, keepdims=True))
    ls_up = ls64[:, imap, :]
    loss_recon = -np.take_along_axis(ls_up, targets[..., None].astype(np.int64), -1).mean()

    bndsum = float(n_segs.sum())
    avg_len = (B * T) / (bndsum + 1e-6)
    l_penalty = 0.01 * (avg_len - 8.0) ** 2
    p0 = np.clip(_sigmoid64(blogit), 1e-6, 1 - 1e-6)
    entropy = -p0 * np.log(p0) - (1 - p0) * np.log(1 - p0)
    loss_entropy = -0.05 * entropy.mean()
    gsum = 0.0
    for b in range(B):
        g_inv = gate_dev[b, min(int(n_segs[b]), S_pad - 1)]
        gsum += gate_dev[b, :n_segs[b]].astype(np.float64).sum() + (T - n_segs[b]) * g_inv
    loss_sparsity = 0.05 * gsum / (B * T)
    loss = 5.0 * loss_recon + l_penalty + loss_sparsity + loss_entropy
    return logits.astype(np.float32), np.float32(loss)


# revision 13
# speedup vs baseline: 1.1418x; 1.1418x over previous
"""Self-contained Trainium2 kernel for nn_CASTGModel_39608188404339.

Host: embedding gather, boundary detection, segment pooling, upsample, loss.
Device (8 NeuronCores, SPMD): the 4-layer jagged transformer over pooled
segments + gate + tied-embedding head.

Sharding: data-parallel over batch (4 elements) x 2-way sequence split per
element. Each core owns TQ = S_pad/2 query tokens of one batch element;
K/V are exchanged within the core pair via AllGather each layer.

Activations live transposed in SBUF: [D on partitions (4x128), tokens free].
All matmuls use float32r (TF32) inputs with fp32 PSUM accumulation.
"""

import contextlib
import ctypes
import math
import sys
import types

import numpy as np


# ---------------------------------------------------------------- ntff shim
def _install_ntff_shim(so_path="/opt/axon/libaxon_pjrt.so"):
    try:
        import antenv.axon_hooks  # noqa: F401
        return
    except ImportError:
        pass
    try:
        lib = ctypes.CDLL(so_path)
    except OSError:
        return
    if not hasattr(lib, "axon_start_nrt_profile"):
        hook = None
    else:
        lib.axon_start_nrt_profile.argtypes = [ctypes.POINTER(ctypes.c_int64), ctypes.c_size_t]
        lib.axon_start_nrt_profile.restype = ctypes.c_int64
        lib.axon_stop_nrt_profile.argtypes = [ctypes.c_char_p]
        lib.axon_stop_nrt_profile.restype = ctypes.c_int64

        @contextlib.contextmanager
        def hook(output_dir, device_ids):
            import jax
            jax.devices()
            if device_ids:
                ids = (ctypes.c_int64 * len(device_ids))(*device_ids)
                rc = lib.axon_start_nrt_profile(ids, len(device_ids))
            else:
                rc = lib.axon_start_nrt_profile(None, 0)
            if rc != 0:
                raise RuntimeError(f"axon_start_nrt_profile rc={rc}")
            try:
                yield
            finally:
                n = lib.axon_stop_nrt_profile(str(output_dir).encode())
                print(f"ntff profile: {n} file(s) written to {output_dir}", file=sys.stderr)

    mod = types.ModuleType("antenv.axon_hooks")
    mod.get_axon_ntff_profile_hook = lambda: hook
    mod.set_axon_ntff_profile_hook = lambda h: None
    import antenv
    antenv.axon_hooks = mod
    sys.modules["antenv.axon_hooks"] = mod


_install_ntff_shim()

import concourse.mybir as mybir  # noqa: E402
import concourse.tile as tile  # noqa: E402
from concourse import bacc  # noqa: E402
from concourse.bass_utils import run_bass_kernel_spmd  # noqa: E402

f32 = mybir.dt.float32
f32r = mybir.dt.float32r
AF = mybir.ActivationFunctionType
ALU = mybir.AluOpType

B, T, V = 4, 2048, 256
D, H, L = 512, 8, 4
DH, FF = D // H, 4 * D
DC = D // 128          # 4 d-chunks
FC = FF // 128         # 16 ff-chunks
NCORES = 8
MASK_BIAS = -60.0

LAST_EXEC_NS = None
_NC_CACHE = {}


def tf32_round(x):
    u = np.ascontiguousarray(x, dtype=np.float32).view(np.uint32)
    r = (u + np.uint32(0x0FFF) + ((u >> np.uint32(13)) & np.uint32(1))) & np.uint32(0xFFFFE000)
    return r.view(np.float32)


def _chunks(total, step=128):
    return [(o, min(step, total - o)) for o in range(0, total, step)]


# ---------------------------------------------------------------- builder
def build(S_pad):
    KC = S_pad // 128
    TQ = S_pad // 2
    HN = TQ // 2
    HALVES = [(0, HN), (HN, TQ - HN)]
    TCS = _chunks(TQ)

    nc = bacc.Bacc("TRN2", target_bir_lowering=False, debug=False, num_devices=NCORES)

    def din(name, shape, dt=f32):
        return nc.dram_tensor(name, shape, dt, kind="ExternalInput").ap()

    zt_d = din("zt", [D, TQ])
    maskb_d = din("maskb", [128, KC])
    wq_d = din("wq", [L, D, D], f32r)
    wk_d = din("wk", [L, D, D], f32r)
    wv_d = din("wv", [L, D, D], f32r)
    bq_d = din("bq", [L, 128, DC])
    bk_d = din("bk", [L, 128, DC])
    bvrep_d = din("bvrep", [L, 128, D])
    wo_d = din("wo", [L, D, D], f32r)
    bo_d = din("bo", [L, 128, DC])
    w1_d = din("w1", [L, D, FF], f32r)
    b1_d = din("b1", [L, 128, FC])
    w2_d = din("w2", [L, FF, D], f32r)
    b2_d = din("b2", [L, 128, DC])
    l1g_d = din("l1g", [L, 128, DC])
    l1b_d = din("l1b", [L, 128, DC])
    l2g_d = din("l2g", [L, 128, DC])
    l2b_d = din("l2b", [L, 128, DC])
    lnfg_d = din("lnfg", [128, DC])
    lnfb_d = din("lnfb", [128, DC])
    embt_d = din("embt", [D, V], f32r)
    bd_d = din("bd", [128, V // 128])
    wg_d = din("wg", [128, DC], f32r)
    bg_d = din("bg", [1, 1])

    lsegt_d = nc.dram_tensor("lsegt", [V, TQ], f32, kind="ExternalOutput").ap()
    gate_d = nc.dram_tensor("gate", [1, TQ], f32, kind="ExternalOutput").ap()

    with tile.TileContext(nc) as tc:
        with (
            tc.tile_pool(name="res", bufs=1) as res,
            tc.tile_pool(name="vec", bufs=2) as vec,
            tc.tile_pool(name="wp", bufs=6) as wp,
            tc.tile_pool(name="sc", bufs=2) as sc,
            tc.tile_pool(name="ex", bufs=4) as ex,
            tc.tile_pool(name="pp", bufs=2, space="PSUM") as pp,
            tc.tile_pool(name="dr", bufs=2, space="DRAM") as dr,
        ):
            # persistent tiles
            xt = res.tile([128, DC * TQ], f32, tag="xt")
            xr = res.tile([128, DC * TQ], f32r, tag="xr")
            xn = res.tile([128, DC * TQ], f32r, tag="xn")
            qt = res.tile([128, DC * TQ], f32r, tag="qt")
            xnf = res.tile([128, DC * S_pad], f32r, tag="xnf")
            ktf = res.tile([128, DC * S_pad], f32r, tag="ktf")
            vext = res.tile([128, KC * 520], f32r, tag="vext")
            ot = res.tile([128, DC * TQ], f32r, tag="ot")
            gb = res.tile([128, TQ], f32, tag="gb")
            lsegs = res.tile([128, (V // 128) * TQ], f32, tag="lsegs")
            maskt = res.tile([128, KC], f32, tag="maskt")
            ones = res.tile([128, 1], f32r, tag="ones")
            epst = res.tile([1, 1], f32, tag="epst")

            nc.vector.memset(ones[:].bitcast(f32), 1.0)
            nc.vector.memset(epst[:], 1e-5)
            # ones columns of vext (written once; v writes only cols 0..63/head)
            for tc_i in range(KC):
                nc.vector.memset(
                    vext[:, tc_i * 520:(tc_i + 1) * 520]
                    .rearrange("p (h x) -> p h x", h=H)[:, :, DH].bitcast(f32),
                    1.0,
                )

            nc.sync.dma_start(maskt[:], maskb_d[:])
            for c in range(DC):
                nc.sync.dma_start(xt[:, c * TQ:(c + 1) * TQ], zt_d[c * 128:(c + 1) * 128, :])
                nc.scalar.copy(xr[:, c * TQ:(c + 1) * TQ], xt[:, c * TQ:(c + 1) * TQ])

            def ln(gvec, bvec):
                """LN over D (partition x chunks) of xt/xr -> xn (f32r)."""
                for ho, hl in HALVES:
                    sq = sc.tile([128, HN], f32r, tag="sq")
                    psm = pp.tile([128, HN], f32, tag="pq", name="psm")[0:1]
                    pss = pp.tile([128, HN], f32, tag="pq", name="pss")[0:1]
                    for c in range(DC):
                        xrc = xr[:, c * TQ + ho: c * TQ + ho + hl]
                        nc.vector.tensor_tensor(sq[:, :hl], xrc, xrc, ALU.mult)
                        nc.tensor.matmul(psm[:, :hl], ones[:], xrc,
                                         start=(c == 0), stop=(c == DC - 1))
                        nc.tensor.matmul(pss[:, :hl], ones[:], sq[:, :hl],
                                         start=(c == 0), stop=(c == DC - 1))
                    A = sc.tile([1, HN], f32, tag="lnA")
                    Bt = sc.tile([1, HN], f32, tag="lnB")
                    nc.vector.tensor_scalar_mul(A[:, :hl], psm[:, :hl], 1.0 / D)
                    nc.vector.tensor_scalar_mul(Bt[:, :hl], pss[:, :hl], 1.0 / D)
                    mub = sc.tile([128, HN], f32, tag="mub")
                    rstdb = sc.tile([128, HN], f32, tag="rstdb")
                    nc.gpsimd.partition_broadcast(mub[:, :hl], A[:, :hl])
                    # A = mu^2; B = var = ms - mu^2; A = sqrt(var+eps); B = rstd
                    nc.vector.tensor_tensor(A[:, :hl], A[:, :hl], A[:, :hl], ALU.mult)
                    nc.vector.tensor_tensor(Bt[:, :hl], Bt[:, :hl], A[:, :hl], ALU.subtract)
                    nc.scalar.activation(A[:, :hl], Bt[:, :hl], AF.Sqrt, bias=epst[:])
                    nc.vector.reciprocal(Bt[:, :hl], A[:, :hl])
                    nc.gpsimd.partition_broadcast(rstdb[:, :hl], Bt[:, :hl])
                    for c in range(DC):
                        s = slice(c * TQ + ho, c * TQ + ho + hl)
                        t1 = sc.tile([128, HN], f32, tag="t1")
                        nc.vector.tensor_tensor(t1[:, :hl], xt[:, s], mub[:, :hl], ALU.subtract)
                        nc.vector.scalar_tensor_tensor(
                            t1[:, :hl], t1[:, :hl], gvec[:, c:c + 1], rstdb[:, :hl],
                            ALU.mult, ALU.mult)
                        nc.vector.tensor_scalar_add(xn[:, s], t1[:, :hl], bvec[:, c:c + 1])

            for l in range(L):
                l1g = vec.tile([128, DC], f32, tag="l1g")
                l1b = vec.tile([128, DC], f32, tag="l1b")
                l2g = vec.tile([128, DC], f32, tag="l2g")
                l2b = vec.tile([128, DC], f32, tag="l2b")
                bqv = vec.tile([128, DC], f32, tag="bqv")
                bkv = vec.tile([128, DC], f32, tag="bkv")
                bov = vec.tile([128, DC], f32, tag="bov")
                b2v = vec.tile([128, DC], f32, tag="b2v")
                b1v = vec.tile([128, FC], f32, tag="b1v")
                bvr = vec.tile([128, D], f32, tag="bvr")
                for t_, d_ in ((l1g, l1g_d), (l1b, l1b_d), (l2g, l2g_d), (l2b, l2b_d),
                               (bqv, bq_d), (bkv, bk_d), (bov, bo_d), (b2v, b2_d),
                               (b1v, b1_d), (bvr, bvrep_d)):
                    nc.sync.dma_start(t_[:], d_[l])

                # ---- LN1
                ln(l1g, l1b)

                # ---- exchange xn halves within the pair (AllGather)
                xn_src = dr.tile([D, TQ], f32, tag="xn_src")
                xn_dst = dr.tile([2, D, TQ], f32, tag="xn_dst")
                for c in range(DC):
                    nc.sync.dma_start(xn_src[c * 128:(c + 1) * 128, :],
                                      xn[:, c * TQ:(c + 1) * TQ].bitcast(f32))
                nc.gpsimd.collective_compute(
                    "AllGather", ALU.bypass,
                    replica_groups=[[0, 1], [2, 3], [4, 5], [6, 7]],
                    ins=[xn_src[:]], outs=[xn_dst[:]])
                for c in range(DC):
                    for half in range(2):
                        nc.sync.dma_start(
                            xnf[:, c * S_pad + half * TQ: c * S_pad + (half + 1) * TQ]
                            .bitcast(f32),
                            xn_dst[half, c * 128:(c + 1) * 128, :])

                # ---- Q projection (overlaps the exchange)
                for kind, wdram, bvec, dst in (("q", wq_d, bqv, qt),):
                    wts = []
                    for c in range(DC):
                        wt = wp.tile([128, D], f32r, tag="wt", name="wt")
                        nc.sync.dma_start(wt[:], wdram[l, c * 128:(c + 1) * 128, :])
                        wts.append(wt)
                    for mc in range(DC):
                        for ho, hl in HALVES:
                            pq = pp.tile([128, HN], f32, tag="pq", name="pq")
                            for c in range(DC):
                                nc.tensor.matmul(
                                    pq[:, :hl], wts[c][:, mc * 128:(mc + 1) * 128],
                                    xn[:, c * TQ + ho: c * TQ + ho + hl],
                                    start=(c == 0), stop=(c == DC - 1))
                            nc.vector.tensor_scalar_add(
                                dst[:, mc * TQ + ho: mc * TQ + ho + hl],
                                pq[:, :hl], bvec[:, mc:mc + 1])

                # ---- K projection over the full sequence -> ktf
                wts_k = []
                for c in range(DC):
                    wt = wp.tile([128, D], f32r, tag="wt", name="wt")
                    nc.sync.dma_start(wt[:], wk_d[l, c * 128:(c + 1) * 128, :])
                    wts_k.append(wt)
                NSP = S_pad // 4
                for mc in range(DC):
                    for qh in range(4):
                        pq = pp.tile([128, HN], f32, tag="pq", name="pq")
                        for c in range(DC):
                            nc.tensor.matmul(
                                pq[:, :NSP],
                                wts_k[c][:, mc * 128:(mc + 1) * 128],
                                xnf[:, c * S_pad + qh * NSP: c * S_pad + (qh + 1) * NSP],
                                start=(c == 0), stop=(c == DC - 1))
                        nc.vector.tensor_scalar_add(
                            ktf[:, mc * S_pad + qh * NSP: mc * S_pad + (qh + 1) * NSP],
                            pq[:, :NSP], bkv[:, mc:mc + 1])

                # ---- V projection over the full sequence -> vext
                wts_v = []
                for c in range(DC):
                    wt = wp.tile([128, D], f32r, tag="wt", name="wt")
                    nc.sync.dma_start(wt[:], wv_d[l, c * 128:(c + 1) * 128, :])
                    wts_v.append(wt)
                for tj in range(KC):
                    pvt = pp.tile([128, D], f32, tag="pq", name="pvt")
                    for c in range(DC):
                        nc.tensor.matmul(
                            pvt[:], xnf[:, c * S_pad + tj * 128: c * S_pad + (tj + 1) * 128],
                            wts_v[c][:], start=(c == 0), stop=(c == DC - 1))
                    nc.vector.tensor_tensor(
                        vext[:, tj * 520:(tj + 1) * 520]
                        .rearrange("p (h x) -> p h x", h=H)[:, :, 0:DH],
                        pvt[:].rearrange("p (h x) -> p h x", h=H),
                        bvr[:].rearrange("p (h x) -> p h x", h=H),
                        ALU.add)

                # ---- attention
                att_cm = tc.tile_pool(name=f"att{l}", bufs=2, space="PSUM")
                att_ps = att_cm.__enter__()
                for h in range(H):
                    prow = (h % 2) * DH
                    cblk = h // 2
                    pos = [att_ps.tile([65, HN], f32, tag="po", name="po") for _ in range(2)]
                    for j in range(KC):
                        pS = att_ps.tile([128, 1024], f32, tag="pS", name="pS")
                        for hi, (ho, hl) in enumerate(HALVES):
                            nc.tensor.matmul(
                                pS[:, hi * 512: hi * 512 + hl],
                                ktf[prow:prow + DH, cblk * S_pad + j * 128: cblk * S_pad + (j + 1) * 128],
                                qt[prow:prow + DH, cblk * TQ + ho: cblk * TQ + ho + hl],
                                start=True, stop=True)
                        et = ex.tile([128, 2 * HN], f32r, tag="et")
                        nc.scalar.activation(
                            et[:].rearrange("p (b x) -> p b x", b=2),
                            pS[:].rearrange("p (b x) -> p b x", b=2)[:, :, 0:HN],
                            AF.Exp, bias=maskt[:, j:j + 1], scale=0.125)
                        for hi, (ho, hl) in enumerate(HALVES):
                            nc.tensor.matmul(
                                pos[hi][:, :hl],
                                vext[:, j * 520 + h * 65: j * 520 + (h + 1) * 65],
                                et[:, hi * HN: hi * HN + hl],
                                start=(j == 0), stop=(j == KC - 1))
                    for hi, (ho, hl) in enumerate(HALVES):
                        po = pos[hi]
                        rec = sc.tile([1, HN], f32, tag="rec")
                        nc.vector.reciprocal(rec[:, :hl], po[DH:DH + 1, :hl])
                        rb = sc.tile([64, HN], f32, tag="rb")
                        nc.gpsimd.partition_broadcast(rb[:, :hl], rec[:, :hl])
                        nc.vector.tensor_tensor(
                            ot[prow:prow + DH, cblk * TQ + ho: cblk * TQ + ho + hl],
                            po[0:DH, :hl], rb[:, :hl], ALU.mult)
                att_cm.__exit__(None, None, None)

                # ---- Wo + residual
                wos = []
                for c in range(DC):
                    wt = wp.tile([128, D], f32r, tag="wt")
                    nc.sync.dma_start(wt[:], wo_d[l, c * 128:(c + 1) * 128, :])
                    wos.append(wt)
                for mc in range(DC):
                    for ho, hl in HALVES:
                        pw = pp.tile([128, HN], f32, tag="pq")
                        for c in range(DC):
                            nc.tensor.matmul(
                                pw[:, :hl], wos[c][:, mc * 128:(mc + 1) * 128],
                                ot[:, c * TQ + ho: c * TQ + ho + hl],
                                start=(c == 0), stop=(c == DC - 1))
                        s = slice(mc * TQ + ho, mc * TQ + ho + hl)
                        nc.vector.scalar_tensor_tensor(
                            xt[:, s], pw[:, :hl], bov[:, mc:mc + 1], xt[:, s],
                            ALU.add, ALU.add)
                    nc.scalar.copy(xr[:, mc * TQ:(mc + 1) * TQ],
                                   xt[:, mc * TQ:(mc + 1) * TQ])

                # ---- LN2
                ln(l2g, l2b)

                # ---- FFN (fused FF1 -> gelu -> FF2 accumulation, per token half)
                ffn_cm = tc.tile_pool(name=f"ffn{l}", bufs=4, space="PSUM")
                ffn_ps = ffn_cm.__enter__()
                for ho, hl in HALVES:
                    accs = [ffn_ps.tile([128, HN], f32, tag="pacc", name=f"acc{mc}")
                            for mc in range(DC)]
                    for fc in range(FC):
                        w1ts = []
                        for c in range(DC):
                            w1t = wp.tile([128, 128], f32r, tag="w1t", name="w1t")
                            nc.sync.dma_start(
                                w1t[:], w1_d[l, c * 128:(c + 1) * 128,
                                             fc * 128:(fc + 1) * 128])
                            w1ts.append(w1t)
                        w2t = wp.tile([128, D], f32r, tag="w2t", name="w2t")
                        nc.sync.dma_start(w2t[:], w2_d[l, fc * 128:(fc + 1) * 128, :])
                        pf = pp.tile([128, HN], f32, tag="pq", name="pf")
                        for c in range(DC):
                            nc.tensor.matmul(
                                pf[:, :hl], w1ts[c][:],
                                xn[:, c * TQ + ho: c * TQ + ho + hl],
                                start=(c == 0), stop=(c == DC - 1))
                        fft = ex.tile([128, HN], f32r, tag="fft", name="fft")
                        nc.scalar.activation(fft[:, :hl], pf[:, :hl],
                                             AF.Gelu_apprx_tanh, bias=b1v[:, fc:fc + 1])
                        for mc in range(DC):
                            nc.tensor.matmul(
                                accs[mc][:, :hl], w2t[:, mc * 128:(mc + 1) * 128],
                                fft[:, :hl], start=(fc == 0), stop=(fc == FC - 1))
                    for mc in range(DC):
                        s_ = slice(mc * TQ + ho, mc * TQ + ho + hl)
                        nc.vector.scalar_tensor_tensor(
                            xt[:, s_], accs[mc][:, :hl], b2v[:, mc:mc + 1], xt[:, s_],
                            ALU.add, ALU.add)
                        if ho != 0:
                            nc.scalar.copy(xr[:, mc * TQ:(mc + 1) * TQ],
                                           xt[:, mc * TQ:(mc + 1) * TQ])
                ffn_cm.__exit__(None, None, None)

            # ---- head: gate
            wgt = vec.tile([128, DC], f32r, tag="wgt")
            bgt = vec.tile([1, 1], f32, tag="bgt")
            lnfg = vec.tile([128, DC], f32, tag="lnfg")
            lnfb = vec.tile([128, DC], f32, tag="lnfb")
            bdv = vec.tile([128, V // 128], f32, tag="bdv")
            nc.sync.dma_start(wgt[:], wg_d[:])
            nc.sync.dma_start(bgt[:], bg_d[:])
            nc.sync.dma_start(lnfg[:], lnfg_d[:])
            nc.sync.dma_start(lnfb[:], lnfb_d[:])
            nc.sync.dma_start(bdv[:], bd_d[:])
            gatet = res.tile([1, TQ], f32, tag="gatet")
            for ho, hl in HALVES:
                pg = pp.tile([128, HN], f32, tag="pq", name="pg")[0:1]
                for c in range(DC):
                    nc.tensor.matmul(pg[:, :hl], wgt[:, c:c + 1],
                                     xr[:, c * TQ + ho: c * TQ + ho + hl],
                                     start=(c == 0), stop=(c == DC - 1))
                nc.scalar.activation(gatet[:, ho:ho + hl], pg[:, :hl], AF.Sigmoid,
                                     bias=bgt[:])
            nc.sync.dma_start(gate_d[:], gatet[:])
            nc.gpsimd.partition_broadcast(gb[:], gatet[:])
            for c in range(DC):
                s = slice(c * TQ, (c + 1) * TQ)
                nc.vector.tensor_tensor(xt[:, s], xt[:, s], gb[:], ALU.mult)
                nc.scalar.copy(xr[:, s], xt[:, s])

            # ---- head: LNf + logits
            ln(lnfg, lnfb)
            embts = []
            for c in range(DC):
                wt = wp.tile([128, V], f32r, tag="embt")
                nc.sync.dma_start(wt[:], embt_d[c * 128:(c + 1) * 128, :])
                embts.append(wt)
            for vc in range(V // 128):
                for ho, hl in HALVES:
                    pl = pp.tile([128, HN], f32, tag="pq")
                    for c in range(DC):
                        nc.tensor.matmul(
                            pl[:, :hl], embts[c][:, vc * 128:(vc + 1) * 128],
                            xn[:, c * TQ + ho: c * TQ + ho + hl],
                            start=(c == 0), stop=(c == DC - 1))
                    nc.vector.tensor_scalar_add(
                        lsegs[:, vc * TQ + ho: vc * TQ + ho + hl], pl[:, :hl],
                        bdv[:, vc:vc + 1])
            for vc in range(V // 128):
                nc.sync.dma_start(lsegt_d[vc * 128:(vc + 1) * 128, :],
                                  lsegs[:, vc * TQ:(vc + 1) * TQ])

    nc.compile()
    return nc


def _get_nc(S_pad):
    if S_pad not in _NC_CACHE:
        _NC_CACHE[S_pad] = build(S_pad)
    return _NC_CACHE[S_pad]


# ---------------------------------------------------------------- host side
def _sigmoid64(x):
    x = x.astype(np.float64)
    return np.where(x >= 0, 1.0 / (1.0 + np.exp(-x)), np.exp(x) / (1.0 + np.exp(x)))


def kernel(idx, targets, params):
    global LAST_EXEC_NS
    idx = np.asarray(idx)
    targets = np.asarray(targets)
    p = {k: np.asarray(v, dtype=np.float32) for k, v in params.items()}

    embed = p["embed"]                      # [V, D]
    h = embed[idx]                          # [B, T, D] f32
    blogit = (h.reshape(-1, D) @ p["Wb"]).reshape(B, T) + p["bb"][0]
    psig = _sigmoid64(blogit / np.float32(0.5))
    hard = psig > 0.5
    hard[:, 0] = True
    n_segs = hard.sum(1).astype(np.int64)
    S = int(n_segs.max())
    S_pad = min(max(1152, 128 * math.ceil((S + 1) / 128)), 2048)
    KC = S_pad // 128
    TQ = S_pad // 2

    # segment-pool z
    z = np.zeros((B, S_pad, D), np.float32)
    for b in range(B):
        starts = np.flatnonzero(hard[b])
        sums = np.add.reduceat(h[b], starts, axis=0)
        cnts = np.diff(np.append(starts, T)).astype(np.float32)
        z[b, :n_segs[b]] = sums / cnts[:, None]

    # shared (replicated) weight tensors
    Wqkv, bqkv = p["Wqkv"], p["bqkv"]
    shared = {
        "wq": tf32_round(Wqkv[:, :, 0:D]),
        "wk": tf32_round(Wqkv[:, :, D:2 * D]),
        "wv": tf32_round(Wqkv[:, :, 2 * D:3 * D]),
        "bq": np.ascontiguousarray(bqkv[:, 0:D].reshape(L, DC, 128).transpose(0, 2, 1)),
        "bk": np.ascontiguousarray(bqkv[:, D:2 * D].reshape(L, DC, 128).transpose(0, 2, 1)),
        "bvrep": np.ascontiguousarray(
            np.broadcast_to(bqkv[:, None, 2 * D:3 * D], (L, 128, D))),
        "wo": tf32_round(p["Wo"]),
        "bo": np.ascontiguousarray(p["bo"].reshape(L, DC, 128).transpose(0, 2, 1)),
        "w1": tf32_round(p["W1"]),
        "b1": np.ascontiguousarray(p["b1"].reshape(L, FC, 128).transpose(0, 2, 1)),
        "w2": tf32_round(p["W2"]),
        "b2": np.ascontiguousarray(p["b2"].reshape(L, DC, 128).transpose(0, 2, 1)),
        "l1g": np.ascontiguousarray(p["ln1_g"].reshape(L, DC, 128).transpose(0, 2, 1)),
        "l1b": np.ascontiguousarray(p["ln1_b"].reshape(L, DC, 128).transpose(0, 2, 1)),
        "l2g": np.ascontiguousarray(p["ln2_g"].reshape(L, DC, 128).transpose(0, 2, 1)),
        "l2b": np.ascontiguousarray(p["ln2_b"].reshape(L, DC, 128).transpose(0, 2, 1)),
        "lnfg": np.ascontiguousarray(p["lnf_g"].reshape(DC, 128).T),
        "lnfb": np.ascontiguousarray(p["lnf_b"].reshape(DC, 128).T),
        "embt": tf32_round(np.ascontiguousarray(embed.T)),
        "bd": np.ascontiguousarray(p["bd"].reshape(V // 128, 128).T),
        "wg": tf32_round(np.ascontiguousarray(p["Wg"].reshape(DC, 128).T)),
        "bg": p["bg"].reshape(1, 1),
    }

    in_maps = []
    for c in range(NCORES):
        b = c // 2
        lo = (c % 2) * TQ
        m = dict(shared)
        m["zt"] = np.ascontiguousarray(z[b, lo:lo + TQ, :].T)
        kmask = np.where(np.arange(S_pad) < n_segs[b], 0.0, MASK_BIAS).astype(np.float32)
        m["maskb"] = np.ascontiguousarray(kmask.reshape(KC, 128).T)
        in_maps.append(m)

    nc = _get_nc(S_pad)
    import os
    trace = bool(os.environ.get("BASS_KERNEL_TRACE"))
    res = run_bass_kernel_spmd(nc, in_maps, list(range(NCORES)), trace=trace)
    if trace:
        LAST_EXEC_NS = res.exec_time_ns

    lseg = np.zeros((B, S_pad, V), np.float32)
    gate_dev = np.zeros((B, S_pad), np.float32)
    for c in range(NCORES):
        b = c // 2
        lo = (c % 2) * TQ
        lseg[b, lo:lo + TQ] = res.results[c]["lsegt"].T
        gate_dev[b, lo:lo + TQ] = res.results[c]["gate"][0]

    # upsample + loss (host, f64)
    imap = (np.arange(T) * S) // T
    logits = lseg[:, imap, :]
    ls64 = lseg[:, :S, :].astype(np.float64)
    ls64 = ls64 - ls64.max(-1, keepdims=True)
    ls64 = ls64 - np.log(np.exp(ls64).sum(-1, keepdims=True))
    ls_up = ls64[:, imap, :]
    loss_recon = -np.take_along_axis(ls_up, targets[..., None].astype(np.int64), -1).mean()

    bndsum = float(n_segs.sum())
    avg_len = (B * T) / (bndsum + 1e-6)
    l_penalty = 0.01 * (avg_len - 8.0) ** 2
    p0 = np.clip(_sigmoid64(blogit), 1e-6, 1 - 1e-6)
    entropy = -p0 * np.log(p0) - (1 - p0) * np.log(1 - p0)
    loss_entropy = -0.05 * entropy.mean()
    gsum = 0.0
    for b in range(B):
        g_inv = gate_dev[b, min(int(n_segs[b]), S_pad - 1)]
        gsum += gate_dev[b, :n_segs[b]].astype(np.float64).sum() + (T - n_segs[b]) * g_inv
    loss_sparsity = 0.05 * gsum / (B * T)
    loss = 5.0 * loss_recon + l_penalty + loss_sparsity + loss_entropy
    return logits.astype(np.float32), np.float32(loss)
